# revision 27
# baseline (speedup 1.0000x reference)
"""LoRA multi-head attention on 8 TRN2 NeuronCores.

Sharding: data-parallel over batch (B=8 -> 1 batch element per core),
weights replicated, no collectives.

Host side (in kernel()): inputs are cast to bf16 and pre-transposed so
the device reads exactly the layouts the TensorEngine needs (the
contraction dim on partitions). Device DMA on this system is slow
(~30-100 GB/s aggregate), so shipping 12 MB of ready-to-use bf16
beats 40 MB of on-device cast/transpose traffic by a wide margin.

Device side per core, all bf16 with fp32 PSUM accumulation:
  qT = (WqT.T @ xT + BqT.T (AqT.T xT) / 16) / 8     [dout, n]
  kT likewise; v natural [n, dout] via (xT.T @ WvT), stored per-head
  with a ones column ([v_h | 1]) so PV also yields softmax denoms.
  Per head: S^T = kT_h.T qT_h -> exp (no max-sub; |s|=O(4)) -> PV;
  normalize via ones-outer-product broadcast + fast reciprocal.
  out = attnT.T @ WoT + lora + bo (bias via K=1 ones matmul).
"""

import sys

if "/opt/trn_rl_repo" not in sys.path:
    sys.path.insert(0, "/opt/trn_rl_repo")

import numpy as np
import ml_dtypes

BF16 = ml_dtypes.bfloat16

N = 1024  # tokens
D = 1024  # model dim
H = 16    # heads
HD = 64   # head dim
R = 16    # lora rank
P = 128   # partitions
F = 512   # psum free-dim tile
NCORES = 8
SCALING = 1.0 / 16.0  # lora alpha/rank
SCALE = HD ** -0.5

_CACHE = {}


def _build():
    import concourse.bacc as bacc
    import concourse.mybir as mybir
    import concourse.tile as tile

    f32 = mybir.dt.float32
    bf16 = mybir.dt.bfloat16
    Exp = mybir.ActivationFunctionType.Exp

    nc = bacc.Bacc("TRN2", target_bir_lowering=False, debug=False)

    # all big params arrive pre-transposed, bf16, from the host
    xT_e = nc.declare_dram_parameter("xT", [D, N], bf16, isOutput=False)
    wT_e = {
        nm: nc.declare_dram_parameter(nm, [D, D], bf16, isOutput=False)
        for nm in ("WqT", "WkT", "WvT", "WoT")
    }
    aT_e = {
        nm: nc.declare_dram_parameter(nm, [D, R], bf16, isOutput=False)
        for nm in ("AqT", "AkT", "AvT", "AoT")
    }
    bT_e = {
        nm: nc.declare_dram_parameter(nm, [R, D], bf16, isOutput=False)
        for nm in ("BqT", "BkT", "BvT", "BoT")
    }
    bo_e = nc.declare_dram_parameter("bo", [1, D], bf16, isOutput=False)
    out_e = nc.declare_dram_parameter("out", [N, D], f32, isOutput=True)

    with tile.TileContext(nc) as tc:
        with (
            tc.tile_pool(name="wpool", bufs=1) as wpool,
            tc.tile_pool(name="stage", bufs=2) as stage,
            tc.tile_pool(name="ps", bufs=1, space="PSUM") as ps,
        ):
            qs = [nc.sync, nc.scalar, nc.gpsimd]

            # ---- load pre-transposed tensors straight into SBUF ----
            T = {}
            qi = 0
            for nm, ext in [("x", xT_e), ("Wv", wT_e["WvT"]),
                            ("Wq", wT_e["WqT"]), ("Wk", wT_e["WkT"]),
                            ("Wo", wT_e["WoT"])]:
                T[nm] = []
                for t in range(8):
                    tt = wpool.tile([P, D], bf16, tag=f"T_{nm}_{t}",
                                    name=f"T_{nm}_{t}")
                    qs[qi % 3].dma_start(out=tt[:],
                                         in_=ext[t * P:(t + 1) * P, :])
                    qi += 1
                    T[nm].append(tt)
            aT = {}
            for nm, ext in aT_e.items():
                key = nm[:2]  # AqT -> Aq
                aT[key] = []
                for t in range(8):
                    tt = wpool.tile([P, R], bf16, tag=f"aT_{nm}_{t}",
                                    name=f"aT_{nm}_{t}")
                    qs[qi % 3].dma_start(out=tt[:],
                                         in_=ext[t * P:(t + 1) * P, :])
                    qi += 1
                    aT[key].append(tt)
            bT = {}
            for nm, ext in bT_e.items():
                key = nm[:2]  # BqT -> Bq
                tt = wpool.tile([R, D], bf16, tag=f"bT_{nm}")
                qs[qi % 3].dma_start(out=tt[:], in_=ext[:, :])
                qi += 1
                bT[key] = tt
            bo_sb = wpool.tile([1, D], bf16, tag="bo")
            nc.sync.dma_start(out=bo_sb[:], in_=bo_e[:, :])
            ones128 = wpool.tile([1, P], bf16, tag="ones128")
            nc.vector.memset(ones128[:], 1.0)
            onesf = wpool.tile([P, HD], f32, tag="onesf")
            nc.vector.memset(onesf[:], 1.0)

            # ---- PE warm-up, gated on the last x tile so it runs right
            # before dense compute (brings HAM to K=8/8) ----
            wps = ps.tile([P, F], f32, tag="pvpsum", bufs=2)
            for _ in range(24):
                nc.tensor.matmul(wps[:], T["x"][7][:, 0:P],
                                 T["x"][7][:, 0:F], start=True, stop=True)

            # ---- lora intermediates (need only xT + A^T) ----
            tv = wpool.tile([R, D], bf16, tag="tvT")
            for nh in range(2):
                ns = slice(nh * F, (nh + 1) * F)
                pt = ps.tile([R, F], f32, tag="pvpsum", bufs=2)
                for kt in range(8):
                    nc.tensor.matmul(pt[:], aT["Av"][kt][:],
                                     T["x"][kt][:, ns],
                                     start=(kt == 0), stop=(kt == 7))
                nc.vector.tensor_scalar_mul(tv[:, ns], pt[:], SCALING)
            tsb = {}
            for nm, anm in (("q", "Aq"), ("k", "Ak")):
                for nh in range(2):
                    ns = slice(nh * F, (nh + 1) * F)
                    pt = ps.tile([R, F], f32, tag="pvpsum", bufs=2)
                    for kt in range(8):
                        nc.tensor.matmul(pt[:], aT[anm][kt][:],
                                         T["x"][kt][:, ns],
                                         start=(kt == 0), stop=(kt == 7))
                    t_s = stage.tile([R, F], bf16, tag="tsb", bufs=4,
                                     name=f"tsb_{nm}_{nh}")
                    nc.vector.tensor_scalar_mul(t_s[:], pt[:], SCALING)
                    tsb[(nm, nh)] = t_s

            # ---- v natural, per-head layout [v_h | 1], with the dt=0
            # projection woven in so attention starts immediately after ----
            qks = {}

            def proj_gen(dt):
                qk = {}
                for nm, wnm, bnm, scl in (("q", "Wq", "Bq", SCALE),
                                          ("k", "Wk", "Bk", None)):
                    dst = wpool.tile([P, D], bf16, tag=f"{nm}T",
                                     bufs=3, name=f"{nm}T_{dt}")
                    qk[nm] = dst
                    pq = ps.tile([P, 2 * F], f32, tag="projpair", bufs=1)
                    for kt in range(8):
                        wblk = T[wnm][kt][:, dt * P:(dt + 1) * P]
                        for nh in range(2):
                            nc.tensor.matmul(
                                pq[:, nh * F:(nh + 1) * F], wblk,
                                T["x"][kt][:, nh * F:(nh + 1) * F],
                                start=(kt == 0), stop=False)
                            yield
                    bblk = bT[bnm][:, dt * P:(dt + 1) * P]
                    for nh in range(2):
                        nc.tensor.matmul(pq[:, nh * F:(nh + 1) * F], bblk,
                                         tsb[(nm, nh)][:],
                                         start=False, stop=True)
                        yield
                    if scl is None:
                        nc.vector.tensor_copy(dst[:], pq[:])
                    else:
                        nc.vector.tensor_scalar_mul(dst[:], pq[:], scl)
                    yield
                qks[dt] = qk

            VW = H * (HD + 1)  # 1040
            v_sb = [wpool.tile([P, VW], bf16, tag=f"v_{t}",
                               name=f"v_{t}") for t in range(8)]
            g0 = proj_gen(0)
            for nt in range(8):
                vr = v_sb[nt][:].rearrange("p (h c) -> p h c", c=HD + 1)
                pv = ps.tile([P, 2 * F], f32, tag="spair", bufs=2)
                for kt in range(8):
                    xblk = T["x"][kt][:, nt * P:(nt + 1) * P]
                    for dh in range(2):
                        nc.tensor.matmul(
                            pv[:, dh * F:(dh + 1) * F], xblk,
                            T["Wv"][kt][:, dh * F:(dh + 1) * F],
                            start=(kt == 0), stop=False)
                    next(g0, None)
                for dh in range(2):
                    nc.tensor.matmul(pv[:, dh * F:(dh + 1) * F],
                                     tv[:, nt * P:(nt + 1) * P],
                                     bT["Bv"][:, dh * F:(dh + 1) * F],
                                     start=False, stop=True)
                pvr = pv[:].rearrange("p (h c) -> p h c", c=HD)
                nc.vector.tensor_copy(vr[:, :, 0:HD], pvr[:])
                nc.vector.memset(vr[:, :, HD:HD + 1], 1.0)
                for _ in range(2):
                    next(g0, None)
            for _ in g0:
                pass

            # ---- per dout-tile: qT, kT, then its 2 heads' attention.
            # The NEXT tile's projection matmuls are woven into the
            # attention inner loop (generator) so the PE stays dense
            # while ACT runs the exps -- keeps HAM at K=8/8. ----
            attnT = [wpool.tile([P, D], bf16, tag=f"attnT_{t}",
                                name=f"attnT_{t}") for t in range(8)]
            for dt in range(8):
                g = proj_gen(dt + 1) if dt < 7 else iter(())
                h0 = 2 * dt
                qt = qks[dt]["q"]
                ktt = qks[dt]["k"]
                for nh in range(2):
                    ns = slice(nh * F, (nh + 1) * F)
                    po = {}
                    for h in (h0, h0 + 1):
                        po[h] = ps.tile([HD + 1, F], f32, tag="pvpsum",
                                        bufs=2, name=f"po_{h}_{nh}")
                    for mt in range(8):
                        spair = ps.tile([P, 2 * F], f32, tag="spair",
                                        bufs=2)
                        for hi, h in enumerate((h0, h0 + 1)):
                            ro = (h % 2) * HD
                            m0 = mt * P
                            nc.tensor.matmul(
                                spair[:, hi * F:(hi + 1) * F],
                                ktt[ro:ro + HD, m0:m0 + P],
                                qt[ro:ro + HD, ns], start=True, stop=True)
                        pte = stage.tile([P, 2 * F], bf16, tag="pt", bufs=2)
                        nc.scalar.activation(pte[:], spair[:], Exp)
                        for hi, h in enumerate((h0, h0 + 1)):
                            nc.tensor.matmul(
                                po[h][:],
                                v_sb[mt][:, h * (HD + 1):(h + 1) * (HD + 1)],
                                pte[:, hi * F:(hi + 1) * F],
                                start=(mt == 0), stop=(mt == 7))
                        for _ in range(3):
                            next(g, None)
                    for h in (h0, h0 + 1):
                        ro = (h % 2) * HD
                        oah = stage.tile([HD + 1, F], f32, tag="oah")
                        nc.vector.tensor_copy(oah[:], po[h][:])
                        pb = ps.tile([HD, F], f32, tag="pvpsum", bufs=2)
                        nc.tensor.matmul(pb[:], onesf[HD:HD + 1, :],
                                         oah[HD:HD + 1, :],
                                         start=True, stop=True)
                        pbs = stage.tile([HD, F], f32, tag="pbs")
                        nc.vector.reciprocal_approx_fast(pbs[:], pb[:])
                        ast = stage.tile([HD, F], bf16, tag="ast")
                        nc.vector.tensor_mul(ast[:], oah[0:HD, :], pbs[:])
                        nc.sync.dma_start(out=attnT[dt][ro:ro + HD, ns],
                                          in_=ast[:])
                        for _ in range(2):
                            next(g, None)
                for _ in g:
                    pass

            # ---- output projection ----
            to = wpool.tile([R, D], bf16, tag="toT")
            for nh in range(2):
                ns = slice(nh * F, (nh + 1) * F)
                pt = ps.tile([R, F], f32, tag="pvpsum", bufs=2)
                for kt in range(8):
                    nc.tensor.matmul(pt[:], aT["Ao"][kt][:],
                                     attnT[kt][:, ns],
                                     start=(kt == 0), stop=(kt == 7))
                nc.vector.tensor_scalar_mul(to[:, ns], pt[:], SCALING)
            for nt in range(8):
                pf = ps.tile([P, 2 * F], f32, tag="spair", bufs=2)
                for dh in range(2):
                    nc.tensor.matmul(pf[:, dh * F:(dh + 1) * F], ones128[:],
                                     bo_sb[:, dh * F:(dh + 1) * F],
                                     start=True, stop=False)
                for kt in range(8):
                    ablk = attnT[kt][:, nt * P:(nt + 1) * P]
                    for dh in range(2):
                        nc.tensor.matmul(pf[:, dh * F:(dh + 1) * F], ablk,
                                         T["Wo"][kt][:, dh * F:(dh + 1) * F],
                                         start=False, stop=False)
                tblk = to[:, nt * P:(nt + 1) * P]
                for dh in range(2):
                    nc.tensor.matmul(pf[:, dh * F:(dh + 1) * F], tblk,
                                     bT["Bo"][:, dh * F:(dh + 1) * F],
                                     start=False, stop=True)
                osb = stage.tile([P, 2 * F], f32, tag="osb")
                nc.vector.tensor_copy(osb[:], pf[:])
                nc.sync.dma_start(out=out_e[nt * P:(nt + 1) * P, :],
                                  in_=osb[:])
    nc.compile()
    return nc


def _get_nc():
    if "nc" not in _CACHE:
        _CACHE["nc"] = _build()
    return _CACHE["nc"]


def _prep_shared(inputs):
    def tb(a):  # transpose + bf16, contiguous
        return np.ascontiguousarray(np.asarray(a, np.float32).T.astype(BF16))

    shared = {}
    for nm in ("Wq", "Wk", "Wv", "Wo", "Aq", "Ak", "Av", "Ao",
               "Bq", "Bk", "Bv", "Bo"):
        shared[nm + "T"] = tb(inputs[nm])
    shared["bo"] = np.ascontiguousarray(
        np.asarray(inputs["bo"], np.float32).reshape(1, D).astype(BF16))
    return shared


def kernel(**inputs):
    from concourse import bass_utils

    nc = _get_nc()
    shared = _prep_shared(inputs)
    x = np.asarray(inputs["x"], np.float32)
    in_maps = []
    for i in range(NCORES):
        m = dict(shared)
        m["xT"] = np.ascontiguousarray(x[i].T.astype(BF16))
        in_maps.append(m)
    res = bass_utils.run_bass_kernel_spmd(nc, in_maps,
                                          core_ids=list(range(NCORES)))
    return np.stack([res.results[i]["out"] for i in range(NCORES)], axis=0)


# revision 29
# speedup vs baseline: 1.0030x; 1.0030x over previous
"""LoRA multi-head attention on 8 TRN2 NeuronCores.

Sharding: data-parallel over batch (B=8 -> 1 batch element per core),
weights replicated, no collectives.

Host side (in kernel()): inputs are cast to bf16 and pre-transposed so
the device reads exactly the layouts the TensorEngine needs (the
contraction dim on partitions). Device DMA on this system is slow
(~30-100 GB/s aggregate), so shipping 12 MB of ready-to-use bf16
beats 40 MB of on-device cast/transpose traffic by a wide margin.

Device side per core, all bf16 with fp32 PSUM accumulation:
  qT = (WqT.T @ xT + BqT.T (AqT.T xT) / 16) / 8     [dout, n]
  kT likewise; v natural [n, dout] via (xT.T @ WvT), stored per-head
  with a ones column ([v_h | 1]) so PV also yields softmax denoms.
  Per head: S^T = kT_h.T qT_h -> exp (no max-sub; |s|=O(4)) -> PV;
  normalize via ones-outer-product broadcast + fast reciprocal.
  out = attnT.T @ WoT + lora + bo (bias via K=1 ones matmul).
"""

import sys

if "/opt/trn_rl_repo" not in sys.path:
    sys.path.insert(0, "/opt/trn_rl_repo")

import numpy as np
import ml_dtypes

BF16 = ml_dtypes.bfloat16

N = 1024  # tokens
D = 1024  # model dim
H = 16    # heads
HD = 64   # head dim
R = 16    # lora rank
P = 128   # partitions
F = 512   # psum free-dim tile
NCORES = 8
SCALING = 1.0 / 16.0  # lora alpha/rank
SCALE = HD ** -0.5

_CACHE = {}


def _build():
    import concourse.bacc as bacc
    import concourse.mybir as mybir
    import concourse.tile as tile

    f32 = mybir.dt.float32
    bf16 = mybir.dt.bfloat16
    Exp = mybir.ActivationFunctionType.Exp

    nc = bacc.Bacc("TRN2", target_bir_lowering=False, debug=False)

    # all big params arrive pre-transposed, bf16, from the host
    xT_e = nc.declare_dram_parameter("xT", [D, N], bf16, isOutput=False)
    wT_e = {
        nm: nc.declare_dram_parameter(nm, [D, D], bf16, isOutput=False)
        for nm in ("WqT", "WkT", "WvT", "WoT")
    }
    aT_e = {
        nm: nc.declare_dram_parameter(nm, [D, R], bf16, isOutput=False)
        for nm in ("AqT", "AkT", "AvT", "AoT")
    }
    bT_e = {
        nm: nc.declare_dram_parameter(nm, [R, D], bf16, isOutput=False)
        for nm in ("BqT", "BkT", "BvT", "BoT")
    }
    bo_e = nc.declare_dram_parameter("bo", [1, D], bf16, isOutput=False)
    out_e = nc.declare_dram_parameter("out", [N, D], f32, isOutput=True)

    with tile.TileContext(nc) as tc:
        with (
            tc.tile_pool(name="wpool", bufs=1) as wpool,
            tc.tile_pool(name="stage", bufs=2) as stage,
            tc.tile_pool(name="ps", bufs=1, space="PSUM") as ps,
        ):
            qs = [nc.sync, nc.scalar, nc.gpsimd]

            # ---- load pre-transposed tensors straight into SBUF ----
            T = {}
            qi = 0
            for nm, ext in [("x", xT_e), ("Wv", wT_e["WvT"]),
                            ("Wq", wT_e["WqT"]), ("Wk", wT_e["WkT"]),
                            ("Wo", wT_e["WoT"])]:
                T[nm] = []
                for t in range(8):
                    tt = wpool.tile([P, D], bf16, tag=f"T_{nm}_{t}",
                                    name=f"T_{nm}_{t}")
                    qs[qi % 3].dma_start(out=tt[:],
                                         in_=ext[t * P:(t + 1) * P, :])
                    qi += 1
                    T[nm].append(tt)
            aT = {}
            for nm, ext in aT_e.items():
                key = nm[:2]  # AqT -> Aq
                aT[key] = []
                for t in range(8):
                    tt = wpool.tile([P, R], bf16, tag=f"aT_{nm}_{t}",
                                    name=f"aT_{nm}_{t}")
                    qs[qi % 3].dma_start(out=tt[:],
                                         in_=ext[t * P:(t + 1) * P, :])
                    qi += 1
                    aT[key].append(tt)
            bT = {}
            for nm, ext in bT_e.items():
                key = nm[:2]  # BqT -> Bq
                tt = wpool.tile([R, D], bf16, tag=f"bT_{nm}")
                qs[qi % 3].dma_start(out=tt[:], in_=ext[:, :])
                qi += 1
                bT[key] = tt
            bo_sb = wpool.tile([1, D], bf16, tag="bo")
            nc.sync.dma_start(out=bo_sb[:], in_=bo_e[:, :])
            ones128 = wpool.tile([1, P], bf16, tag="ones128")
            nc.vector.memset(ones128[:], 1.0)
            onesf = wpool.tile([P, HD], f32, tag="onesf")
            nc.vector.memset(onesf[:], 1.0)

            # ---- PE warm-up, gated on the last x tile so it runs right
            # before dense compute (brings HAM to K=8/8) ----
            wps = ps.tile([P, F], f32, tag="tpsum", bufs=1)
            for _ in range(24):
                nc.tensor.matmul(wps[:], T["x"][7][:, 0:P],
                                 T["x"][7][:, 0:F], start=True, stop=True)

            # ---- lora intermediates (need only xT + A^T) ----
            tv = wpool.tile([R, D], bf16, tag="tvT")
            for nh in range(2):
                ns = slice(nh * F, (nh + 1) * F)
                pt = ps.tile([R, F], f32, tag="tpsum", bufs=1)
                for kt in range(8):
                    nc.tensor.matmul(pt[:], aT["Av"][kt][:],
                                     T["x"][kt][:, ns],
                                     start=(kt == 0), stop=(kt == 7))
                nc.vector.tensor_scalar_mul(tv[:, ns], pt[:], SCALING)
            tsb = {}
            for nm, anm in (("q", "Aq"), ("k", "Ak")):
                for nh in range(2):
                    ns = slice(nh * F, (nh + 1) * F)
                    pt = ps.tile([R, F], f32, tag="tpsum", bufs=1)
                    for kt in range(8):
                        nc.tensor.matmul(pt[:], aT[anm][kt][:],
                                         T["x"][kt][:, ns],
                                         start=(kt == 0), stop=(kt == 7))
                    t_s = stage.tile([R, F], bf16, tag="tsb", bufs=4,
                                     name=f"tsb_{nm}_{nh}")
                    nc.vector.tensor_scalar_mul(t_s[:], pt[:], SCALING)
                    tsb[(nm, nh)] = t_s

            # ---- v natural, per-head layout [v_h | 1], with the dt=0
            # projection woven in so attention starts immediately after ----
            qks = {}

            def proj_gen(dt):
                qk = {}
                for nm, wnm, bnm, scl in (("q", "Wq", "Bq", SCALE),
                                          ("k", "Wk", "Bk", None)):
                    dst = wpool.tile([P, D], bf16, tag=f"{nm}T",
                                     bufs=3, name=f"{nm}T_{dt}")
                    qk[nm] = dst
                    for nh in range(2):
                        ns = slice(nh * F, (nh + 1) * F)
                        pq = ps.tile([P, F], f32, tag="projpsum", bufs=1)
                        for kt in range(8):
                            nc.tensor.matmul(
                                pq[:], T[wnm][kt][:, dt * P:(dt + 1) * P],
                                T["x"][kt][:, ns],
                                start=(kt == 0), stop=False)
                            yield
                        nc.tensor.matmul(pq[:],
                                         bT[bnm][:, dt * P:(dt + 1) * P],
                                         tsb[(nm, nh)][:],
                                         start=False, stop=True)
                        yield
                        if scl is None:
                            nc.vector.tensor_copy(dst[:, ns], pq[:])
                        else:
                            nc.vector.tensor_scalar_mul(dst[:, ns],
                                                        pq[:], scl)
                        yield
                qks[dt] = qk

            VW = H * (HD + 1)  # 1040
            v_sb = [wpool.tile([P, VW], bf16, tag=f"v_{t}",
                               name=f"v_{t}") for t in range(8)]
            g0 = proj_gen(0)
            for nt in range(8):
                vr = v_sb[nt][:].rearrange("p (h c) -> p h c", c=HD + 1)
                for dh in range(2):
                    ds = slice(dh * F, (dh + 1) * F)
                    pv = ps.tile([P, F], f32, tag="spair", bufs=2)
                    for kt in range(8):
                        nc.tensor.matmul(
                            pv[:], T["x"][kt][:, nt * P:(nt + 1) * P],
                            T["Wv"][kt][:, ds], start=(kt == 0), stop=False)
                    nc.tensor.matmul(pv[:], tv[:, nt * P:(nt + 1) * P],
                                     bT["Bv"][:, ds], start=False, stop=True)
                    pvr = pv[:].rearrange("p (h c) -> p h c", c=HD)
                    nc.vector.tensor_copy(vr[:, dh * 8:(dh + 1) * 8, 0:HD],
                                          pvr[:])
                    for _ in range(3):
                        next(g0, None)
                nc.vector.memset(vr[:, :, HD:HD + 1], 1.0)
            for _ in g0:
                pass

            # ---- per dout-tile: qT, kT, then its 2 heads' attention.
            # The NEXT tile's projection matmuls are woven into the
            # attention inner loop (generator) so the PE stays dense
            # while ACT runs the exps -- keeps HAM at K=8/8. ----
            attnT = [wpool.tile([P, D], bf16, tag=f"attnT_{t}",
                                name=f"attnT_{t}") for t in range(8)]
            for dt in range(8):
                g = proj_gen(dt + 1) if dt < 7 else iter(())
                h0 = 2 * dt
                qt = qks[dt]["q"]
                ktt = qks[dt]["k"]
                for nh in range(2):
                    ns = slice(nh * F, (nh + 1) * F)
                    po = {}
                    for h in (h0, h0 + 1):
                        po[h] = ps.tile([HD + 1, F], f32, tag="pvpsum",
                                        bufs=2, name=f"po_{h}_{nh}")
                    for mt in range(8):
                        spair = ps.tile([P, 2 * F], f32, tag="spair",
                                        bufs=2)
                        for hi, h in enumerate((h0, h0 + 1)):
                            ro = (h % 2) * HD
                            m0 = mt * P
                            nc.tensor.matmul(
                                spair[:, hi * F:(hi + 1) * F],
                                ktt[ro:ro + HD, m0:m0 + P],
                                qt[ro:ro + HD, ns], start=True, stop=True)
                        pte = stage.tile([P, 2 * F], bf16, tag="pt", bufs=2)
                        nc.scalar.activation(pte[:], spair[:], Exp)
                        for hi, h in enumerate((h0, h0 + 1)):
                            nc.tensor.matmul(
                                po[h][:],
                                v_sb[mt][:, h * (HD + 1):(h + 1) * (HD + 1)],
                                pte[:, hi * F:(hi + 1) * F],
                                start=(mt == 0), stop=(mt == 7))
                        for _ in range(3):
                            next(g, None)
                    for h in (h0, h0 + 1):
                        ro = (h % 2) * HD
                        oah = stage.tile([HD + 1, F], f32, tag="oah")
                        nc.vector.tensor_copy(oah[:], po[h][:])
                        pb = ps.tile([HD, F], f32, tag="tpsum", bufs=1)
                        nc.tensor.matmul(pb[:], onesf[HD:HD + 1, :],
                                         oah[HD:HD + 1, :],
                                         start=True, stop=True)
                        pbs = stage.tile([HD, F], f32, tag="pbs")
                        nc.vector.reciprocal_approx_fast(pbs[:], pb[:])
                        ast = stage.tile([HD, F], bf16, tag="ast")
                        nc.vector.tensor_mul(ast[:], oah[0:HD, :], pbs[:])
                        nc.sync.dma_start(out=attnT[dt][ro:ro + HD, ns],
                                          in_=ast[:])
                        for _ in range(2):
                            next(g, None)
                for _ in g:
                    pass

            # ---- output projection ----
            to = wpool.tile([R, D], bf16, tag="toT")
            for nh in range(2):
                ns = slice(nh * F, (nh + 1) * F)
                pt = ps.tile([R, F], f32, tag="tpsum", bufs=1)
                for kt in range(8):
                    nc.tensor.matmul(pt[:], aT["Ao"][kt][:],
                                     attnT[kt][:, ns],
                                     start=(kt == 0), stop=(kt == 7))
                nc.vector.tensor_scalar_mul(to[:, ns], pt[:], SCALING)
            for nt in range(8):
                for dh in range(2):
                    ds = slice(dh * F, (dh + 1) * F)
                    pf = ps.tile([P, F], f32, tag="spair", bufs=2)
                    nc.tensor.matmul(pf[:], ones128[:], bo_sb[:, ds],
                                     start=True, stop=False)
                    for kt in range(8):
                        nc.tensor.matmul(pf[:],
                                         attnT[kt][:, nt * P:(nt + 1) * P],
                                         T["Wo"][kt][:, ds],
                                         start=False, stop=False)
                    nc.tensor.matmul(pf[:], to[:, nt * P:(nt + 1) * P],
                                     bT["Bo"][:, ds], start=False, stop=True)
                    osb = stage.tile([P, F], f32, tag="osb")
                    nc.vector.tensor_copy(osb[:], pf[:])
                    nc.sync.dma_start(out=out_e[nt * P:(nt + 1) * P, ds],
                                      in_=osb[:])
    nc.compile()
    return nc


def _get_nc():
    if "nc" not in _CACHE:
        _CACHE["nc"] = _build()
    return _CACHE["nc"]


def _prep_shared(inputs):
    def tb(a):  # transpose + bf16, contiguous
        return np.ascontiguousarray(np.asarray(a, np.float32).T.astype(BF16))

    shared = {}
    for nm in ("Wq", "Wk", "Wv", "Wo", "Aq", "Ak", "Av", "Ao",
               "Bq", "Bk", "Bv", "Bo"):
        shared[nm + "T"] = tb(inputs[nm])
    shared["bo"] = np.ascontiguousarray(
        np.asarray(inputs["bo"], np.float32).reshape(1, D).astype(BF16))
    return shared


def kernel(**inputs):
    from concourse import bass_utils

    nc = _get_nc()
    shared = _prep_shared(inputs)
    x = np.asarray(inputs["x"], np.float32)
    in_maps = []
    for i in range(NCORES):
        m = dict(shared)
        m["xT"] = np.ascontiguousarray(x[i].T.astype(BF16))
        in_maps.append(m)
    res = bass_utils.run_bass_kernel_spmd(nc, in_maps,
                                          core_ids=list(range(NCORES)))
    return np.stack([res.results[i]["out"] for i in range(NCORES)], axis=0)


# revision 30
# speedup vs baseline: 1.0509x; 1.0478x over previous
"""LoRA multi-head attention on 8 TRN2 NeuronCores.

Sharding: data-parallel over batch (B=8 -> 1 batch element per core),
weights replicated, no collectives.

Host side (in kernel()): inputs are cast to bf16 and pre-transposed so
the device reads exactly the layouts the TensorEngine needs (the
contraction dim on partitions). Device DMA on this system is slow
(~30-100 GB/s aggregate), so shipping 12 MB of ready-to-use bf16
beats 40 MB of on-device cast/transpose traffic by a wide margin.

Device side per core, all bf16 with fp32 PSUM accumulation:
  qT = (WqT.T @ xT + BqT.T (AqT.T xT) / 16) / 8     [dout, n]
  kT likewise; v natural [n, dout] via (xT.T @ WvT), stored per-head
  with a ones column ([v_h | 1]) so PV also yields softmax denoms.
  Per head: S^T = kT_h.T qT_h -> exp (no max-sub; |s|=O(4)) -> PV;
  normalize via ones-outer-product broadcast + fast reciprocal.
  out = attnT.T @ WoT + lora + bo (bias via K=1 ones matmul).
"""

import sys

if "/opt/trn_rl_repo" not in sys.path:
    sys.path.insert(0, "/opt/trn_rl_repo")

import numpy as np
import ml_dtypes

BF16 = ml_dtypes.bfloat16

N = 1024  # tokens
D = 1024  # model dim
H = 16    # heads
HD = 64   # head dim
R = 16    # lora rank
P = 128   # partitions
F = 512   # psum free-dim tile
NCORES = 8
SCALING = 1.0 / 16.0  # lora alpha/rank
SCALE = HD ** -0.5

_CACHE = {}


def _build():
    import concourse.bacc as bacc
    import concourse.mybir as mybir
    import concourse.tile as tile

    f32 = mybir.dt.float32
    bf16 = mybir.dt.bfloat16
    Exp = mybir.ActivationFunctionType.Exp

    nc = bacc.Bacc("TRN2", target_bir_lowering=False, debug=False)

    # all big params arrive pre-transposed, bf16, from the host
    xT_e = nc.declare_dram_parameter("xT", [D, N], bf16, isOutput=False)
    wT_e = {
        nm: nc.declare_dram_parameter(nm, [D, D], bf16, isOutput=False)
        for nm in ("WqT", "WkT", "WvT", "WoT")
    }
    aT_e = {
        nm: nc.declare_dram_parameter(nm, [D, R], bf16, isOutput=False)
        for nm in ("AqT", "AkT", "AvT", "AoT")
    }
    bT_e = {
        nm: nc.declare_dram_parameter(nm, [R, D], bf16, isOutput=False)
        for nm in ("BqT", "BkT", "BvT", "BoT")
    }
    bo_e = nc.declare_dram_parameter("bo", [1, D], bf16, isOutput=False)
    out_e = nc.declare_dram_parameter("out", [N, D], bf16, isOutput=True)

    with tile.TileContext(nc) as tc:
        with (
            tc.tile_pool(name="wpool", bufs=1) as wpool,
            tc.tile_pool(name="stage", bufs=2) as stage,
            tc.tile_pool(name="ps", bufs=1, space="PSUM") as ps,
        ):
            qs = [nc.sync, nc.scalar, nc.gpsimd]

            # ---- load pre-transposed tensors straight into SBUF ----
            T = {}
            aT = {}
            bT = {}
            qi = 0

            def load_big(nm, ext):
                nonlocal qi
                T[nm] = []
                for t in range(8):
                    tt = wpool.tile([P, D], bf16, tag=f"T_{nm}_{t}",
                                    name=f"T_{nm}_{t}")
                    qs[qi % 3].dma_start(out=tt[:],
                                         in_=ext[t * P:(t + 1) * P, :])
                    qi += 1
                    T[nm].append(tt)

            def load_a(nm):
                nonlocal qi
                key = nm[:2]
                aT[key] = []
                for t in range(8):
                    tt = wpool.tile([P, R], bf16, tag=f"aT_{nm}_{t}",
                                    name=f"aT_{nm}_{t}")
                    qs[qi % 3].dma_start(out=tt[:],
                                         in_=aT_e[nm][t * P:(t + 1) * P, :])
                    qi += 1
                    aT[key].append(tt)

            def load_b(nm):
                nonlocal qi
                tt = wpool.tile([R, D], bf16, tag=f"bT_{nm}")
                qs[qi % 3].dma_start(out=tt[:], in_=bT_e[nm][:, :])
                qi += 1
                bT[nm[:2]] = tt

            load_big("x", xT_e)
            load_a("AvT")
            load_big("Wv", wT_e["WvT"])
            load_a("AqT")
            load_a("AkT")
            load_b("BvT")
            load_big("Wq", wT_e["WqT"])
            load_big("Wk", wT_e["WkT"])
            load_b("BqT")
            load_b("BkT")
            load_big("Wo", wT_e["WoT"])
            load_a("AoT")
            load_b("BoT")
            bo_sb = wpool.tile([1, D], bf16, tag="bo")
            nc.sync.dma_start(out=bo_sb[:], in_=bo_e[:, :])
            ones128 = wpool.tile([1, P], bf16, tag="ones128")
            nc.vector.memset(ones128[:], 1.0)
            onesf = wpool.tile([P, HD], f32, tag="onesf")
            nc.vector.memset(onesf[:], 1.0)

            # ---- PE warm-up, gated on the last x tile so it runs right
            # before dense compute (brings HAM to K=8/8) ----
            wps = ps.tile([P, F], f32, tag="tpsum", bufs=1)
            for _ in range(24):
                nc.tensor.matmul(wps[:], T["x"][7][:, 0:P],
                                 T["x"][7][:, 0:F], start=True, stop=True)

            # ---- lora intermediates (need only xT + A^T) ----
            tv = wpool.tile([R, D], bf16, tag="tvT")
            for nh in range(2):
                ns = slice(nh * F, (nh + 1) * F)
                pt = ps.tile([R, F], f32, tag="tpsum", bufs=1)
                for kt in range(8):
                    nc.tensor.matmul(pt[:], aT["Av"][kt][:],
                                     T["x"][kt][:, ns],
                                     start=(kt == 0), stop=(kt == 7))
                nc.vector.tensor_scalar_mul(tv[:, ns], pt[:], SCALING)
            tsb = {}
            for nm, anm in (("q", "Aq"), ("k", "Ak")):
                for nh in range(2):
                    ns = slice(nh * F, (nh + 1) * F)
                    pt = ps.tile([R, F], f32, tag="tpsum", bufs=1)
                    for kt in range(8):
                        nc.tensor.matmul(pt[:], aT[anm][kt][:],
                                         T["x"][kt][:, ns],
                                         start=(kt == 0), stop=(kt == 7))
                    t_s = stage.tile([R, F], bf16, tag="tsb", bufs=4,
                                     name=f"tsb_{nm}_{nh}")
                    nc.vector.tensor_scalar_mul(t_s[:], pt[:], SCALING)
                    tsb[(nm, nh)] = t_s

            # ---- v natural, per-head layout [v_h | 1], with the dt=0
            # projection woven in so attention starts immediately after ----
            qks = {}

            def proj_gen(dt):
                qk = {}
                for nm, wnm, bnm, scl in (("q", "Wq", "Bq", SCALE),
                                          ("k", "Wk", "Bk", None)):
                    dst = wpool.tile([P, D], bf16, tag=f"{nm}T",
                                     bufs=3, name=f"{nm}T_{dt}")
                    qk[nm] = dst
                    for nh in range(2):
                        ns = slice(nh * F, (nh + 1) * F)
                        pq = ps.tile([P, F], f32, tag="projpsum", bufs=1)
                        for kt in range(8):
                            nc.tensor.matmul(
                                pq[:], T[wnm][kt][:, dt * P:(dt + 1) * P],
                                T["x"][kt][:, ns],
                                start=(kt == 0), stop=False)
                            yield
                        nc.tensor.matmul(pq[:],
                                         bT[bnm][:, dt * P:(dt + 1) * P],
                                         tsb[(nm, nh)][:],
                                         start=False, stop=True)
                        yield
                        if scl is None:
                            nc.vector.tensor_copy(dst[:, ns], pq[:])
                        else:
                            nc.vector.tensor_scalar_mul(dst[:, ns],
                                                        pq[:], scl)
                        yield
                qks[dt] = qk

            VW = H * (HD + 1)  # 1040
            v_sb = [wpool.tile([P, VW], bf16, tag=f"v_{t}",
                               name=f"v_{t}") for t in range(8)]
            g0 = proj_gen(0)
            for nt in range(8):
                vr = v_sb[nt][:].rearrange("p (h c) -> p h c", c=HD + 1)
                for dh in range(2):
                    ds = slice(dh * F, (dh + 1) * F)
                    pv = ps.tile([P, F], f32, tag="spair", bufs=2)
                    for kt in range(8):
                        nc.tensor.matmul(
                            pv[:], T["x"][kt][:, nt * P:(nt + 1) * P],
                            T["Wv"][kt][:, ds], start=(kt == 0), stop=False)
                    nc.tensor.matmul(pv[:], tv[:, nt * P:(nt + 1) * P],
                                     bT["Bv"][:, ds], start=False, stop=True)
                    pvr = pv[:].rearrange("p (h c) -> p h c", c=HD)
                    nc.vector.tensor_copy(vr[:, dh * 8:(dh + 1) * 8, 0:HD],
                                          pvr[:])
                    for _ in range(3):
                        next(g0, None)
                nc.vector.memset(vr[:, :, HD:HD + 1], 1.0)
            for _ in g0:
                pass

            # ---- per dout-tile: qT, kT, then its 2 heads' attention.
            # The NEXT tile's projection matmuls are woven into the
            # attention inner loop (generator) so the PE stays dense
            # while ACT runs the exps -- keeps HAM at K=8/8. ----
            attnT = [wpool.tile([P, D], bf16, tag=f"attnT_{t}",
                                name=f"attnT_{t}") for t in range(8)]
            for dt in range(8):
                g = proj_gen(dt + 1) if dt < 7 else iter(())
                h0 = 2 * dt
                qt = qks[dt]["q"]
                ktt = qks[dt]["k"]
                for nh in range(2):
                    ns = slice(nh * F, (nh + 1) * F)
                    po = {}
                    for h in (h0, h0 + 1):
                        po[h] = ps.tile([HD + 1, F], f32, tag="pvpsum",
                                        bufs=2, name=f"po_{h}_{nh}")
                    for mt in range(8):
                        spair = ps.tile([P, 2 * F], f32, tag="spair",
                                        bufs=2)
                        for hi, h in enumerate((h0, h0 + 1)):
                            ro = (h % 2) * HD
                            m0 = mt * P
                            nc.tensor.matmul(
                                spair[:, hi * F:(hi + 1) * F],
                                ktt[ro:ro + HD, m0:m0 + P],
                                qt[ro:ro + HD, ns], start=True, stop=True)
                        pte = stage.tile([P, 2 * F], bf16, tag="pt", bufs=2)
                        nc.scalar.activation(pte[:], spair[:], Exp)
                        for hi, h in enumerate((h0, h0 + 1)):
                            nc.tensor.matmul(
                                po[h][:],
                                v_sb[mt][:, h * (HD + 1):(h + 1) * (HD + 1)],
                                pte[:, hi * F:(hi + 1) * F],
                                start=(mt == 0), stop=(mt == 7))
                        for _ in range(3):
                            next(g, None)
                    for h in (h0, h0 + 1):
                        ro = (h % 2) * HD
                        oah = stage.tile([HD + 1, F], f32, tag="oah")
                        nc.vector.tensor_copy(oah[:], po[h][:])
                        pb = ps.tile([HD, F], f32, tag="tpsum", bufs=1)
                        nc.tensor.matmul(pb[:], onesf[HD:HD + 1, :],
                                         oah[HD:HD + 1, :],
                                         start=True, stop=True)
                        pbs = stage.tile([HD, F], f32, tag="pbs")
                        nc.vector.reciprocal_approx_fast(pbs[:], pb[:])
                        ast = stage.tile([HD, F], bf16, tag="ast")
                        nc.vector.tensor_mul(ast[:], oah[0:HD, :], pbs[:])
                        nc.sync.dma_start(out=attnT[dt][ro:ro + HD, ns],
                                          in_=ast[:])
                        for _ in range(2):
                            next(g, None)
                for _ in g:
                    pass

            # ---- output projection ----
            to = wpool.tile([R, D], bf16, tag="toT")
            for nh in range(2):
                ns = slice(nh * F, (nh + 1) * F)
                pt = ps.tile([R, F], f32, tag="tpsum", bufs=1)
                for kt in range(8):
                    nc.tensor.matmul(pt[:], aT["Ao"][kt][:],
                                     attnT[kt][:, ns],
                                     start=(kt == 0), stop=(kt == 7))
                nc.vector.tensor_scalar_mul(to[:, ns], pt[:], SCALING)
            for nt in range(8):
                for dh in range(2):
                    ds = slice(dh * F, (dh + 1) * F)
                    pf = ps.tile([P, F], f32, tag="spair", bufs=2)
                    nc.tensor.matmul(pf[:], ones128[:], bo_sb[:, ds],
                                     start=True, stop=False)
                    for kt in range(8):
                        nc.tensor.matmul(pf[:],
                                         attnT[kt][:, nt * P:(nt + 1) * P],
                                         T["Wo"][kt][:, ds],
                                         start=False, stop=False)
                    nc.tensor.matmul(pf[:], to[:, nt * P:(nt + 1) * P],
                                     bT["Bo"][:, ds], start=False, stop=True)
                    osb = stage.tile([P, F], bf16, tag="osb")
                    nc.vector.tensor_copy(osb[:], pf[:])
                    nc.sync.dma_start(out=out_e[nt * P:(nt + 1) * P, ds],
                                      in_=osb[:])
    nc.compile()
    return nc


def _get_nc():
    if "nc" not in _CACHE:
        _CACHE["nc"] = _build()
    return _CACHE["nc"]


def _prep_shared(inputs):
    def tb(a):  # transpose + bf16, contiguous
        return np.ascontiguousarray(np.asarray(a, np.float32).T.astype(BF16))

    shared = {}
    for nm in ("Wq", "Wk", "Wv", "Wo", "Aq", "Ak", "Av", "Ao",
               "Bq", "Bk", "Bv", "Bo"):
        shared[nm + "T"] = tb(inputs[nm])
    shared["bo"] = np.ascontiguousarray(
        np.asarray(inputs["bo"], np.float32).reshape(1, D).astype(BF16))
    return shared


def kernel(**inputs):
    from concourse import bass_utils

    nc = _get_nc()
    shared = _prep_shared(inputs)
    x = np.asarray(inputs["x"], np.float32)
    in_maps = []
    for i in range(NCORES):
        m = dict(shared)
        m["xT"] = np.ascontiguousarray(x[i].T.astype(BF16))
        in_maps.append(m)
    res = bass_utils.run_bass_kernel_spmd(nc, in_maps,
                                          core_ids=list(range(NCORES)))
    return np.stack([np.asarray(res.results[i]["out"]).astype(np.float32)
                     for i in range(NCORES)], axis=0)
